# revision 15
# baseline (speedup 1.0000x reference)
"""Trainium2 8-core kernel for sample_wise_recolor (retrieval KNN).

Strategy
--------
Both KNN passes (tgt->pred "backward", pred->tgt "forward") are 1-NN-with-ties
problems: the reference only consumes the entries of the top-k that equal the
row minimum.  Queries are binned into equal-count 3D cells of 128 points
(a "block"); each block only scans the db points inside its bounding box
inflated by a safety radius R (any point outside differs > R in x, y or z,
hence d2 > R^2, which is checked and falls back to a full scan).

Device (8 NeuronCores, SPMD, blocks sharded contiguously): per-block
candidate slabs are cut into 256-wide tiles and packed into uniform
8-slot "waves" (one PSUM half = 4 banks = 8 x 256 fp32).  Each wave's DMA
carries both the db tile features and the owning block's query features
(so the slot->block mapping lives in the data, keeping the program
SPMD-identical across cores).  Per wave:
  PE  : 8 matmuls d2 = [qq,1,-2qx,-2qy,-2qz] . [1,pp,px,py,pz]  (K=5 fp32),
        4-band tile_position layout so up to 4 tiles stream concurrently
  DVE : one tensor_reduce min over the 2048-col half -> per-slot row minima
PSUM halves / slab buffers / tm buffers are multi-buffered so DMA, PE and
DVE pipeline across waves.

Host: for each row, the winning tile(s) (within DELTA of the device row-min)
are recomputed *bit-exactly* in the reference's own fp32 rounding (the XLA-CPU
recipe was reverse-engineered and verified: query squares fma(z,z,fma(x,x,
fl(y^2))), db squares fl(fl(x^2+y^2)+z^2), dot = fma chain, combine
fl(fl(qq+pp)-2B)).  This yields the exact row minimum and exact tie set, from
which the reference's scatter/divide/fallback epilogue is reproduced.
"""

import os
import sys

for _p in ("/opt/trn_rl_repo", "/root/.axon_site/_ro/trn_rl_repo"):
    if os.path.isdir(_p) and _p not in sys.path:
        sys.path.insert(0, _p)

import numpy as np

M = 65536          # pred points
N = 49152          # tgt points
T = 128            # screen tile width (quarter of a fp32 PSUM bank)
SLOTS = 16         # tiles per wave (= one PSUM half = 4 banks)
HPB = 4            # tiles per bank (bank = 512 fp32 = HPB*T)
WCOL = 4 * T + 4 * 128   # per-band DRAM cols per wave: 4 db tiles + 4 query sets
CORES = 8
NB = 6             # slab DMA buffers
CH = 16            # waves per tm out-DMA chunk
GRID_B = (8, 8, 6)   # backward pass query grid (tgt, 384 blocks)
GRID_F = (8, 8, 8)   # forward pass query grid (pred, 512 blocks)
RADIUS = 6.0       # slab safety radius
DELTA = 0.25       # screen window width (>> device matmul error ~0.03)
EPS_CERT = 0.12    # asserted bound on |device tile min - exact tile min|

_LAST_RESULTS = None  # BassKernelResults of the last device run (for test.py)


def _f32(a):
    return np.asarray(a, dtype=np.float32)


# ----- bit-exact XLA-CPU fp32 arithmetic emulation (verified vs reference) --

def sq_query(a):
    """jnp.sum(qc*qc, axis=1) inside the per-chunk jit: fma(z,z, fma(x,x, fl(y*y)))."""
    x, z = a[..., 0].astype(np.float64), a[..., 2].astype(np.float64)
    s = _f32(a[..., 1] * a[..., 1]).astype(np.float64)
    s = _f32(x * x + s).astype(np.float64)
    return _f32(z * z + s)


def sq_db(a):
    """jnp.sum(db*db, axis=1) standalone kernel: fl(fl(x^2+y^2)+z^2)."""
    return _f32(_f32(_f32(a[..., 0] * a[..., 0]) + _f32(a[..., 1] * a[..., 1]))
                + _f32(a[..., 2] * a[..., 2]))


def d2_rows(q, qq, db3, pp3):
    """Bit-exact pre-clamp d2 for per-row candidate sets.

    q [R,3], qq [R] (sq_query), db3 [R,C,3], pp3 [R,C] (sq_db gathered)."""
    qb = q[:, None, :]
    r = _f32(qb[..., 0] * db3[..., 0]).astype(np.float64)
    r = _f32(qb[..., 1].astype(np.float64) * db3[..., 1].astype(np.float64) + r).astype(np.float64)
    B = _f32(qb[..., 2].astype(np.float64) * db3[..., 2].astype(np.float64) + r)
    return _f32(_f32(qq[:, None] + pp3) - _f32(2.0) * B)


# --------------------------- device program --------------------------------

def _build_program(nwb, nwf):
    import concourse.bass as bass
    import concourse.mybir as mybir

    nc = bass.Bass()
    f32 = mybir.dt.float32
    # matmul input dtype: float32 (2-pass, exact) or float32r (1-pass, lower
    # precision; byte-compatible with fp32 so only the tensor dtype changes)
    mmdt = {"f32": mybir.dt.float32, "f32r": mybir.dt.float32r}[
        os.environ.get("KNN_MMDT", "f32")]
    # per-wave combined slabs, band-grouped: rows 5g:5g+5, cols w*WCOL +
    # [0:T]=db slot g, [T:2T]=db slot g+4, [2T:2T+128]=q slot g,
    # [2T+128:2T+256]=q slot g+4
    dwb = nc.dram_tensor("dwb", [5 * 4, nwb * WCOL], mmdt, kind="ExternalInput")
    dwf = nc.dram_tensor("dwf", [5 * 4, nwf * WCOL], mmdt, kind="ExternalInput")
    tmb = nc.dram_tensor("tmb", [128, nwb * SLOTS], f32, kind="ExternalOutput")
    tmf = nc.dram_tensor("tmf", [128, nwf * SLOTS], f32, kind="ExternalOutput")

    waves = [(0, j) for j in range(nwb)] + [(1, j) for j in range(nwf)]
    chunks = []
    for pi, nw in ((0, nwb), (1, nwf)):
        for s in range(0, nw, CH):
            chunks.append((pi, s, min(CH, nw - s)))
    gw = lambda pi, j: j + (nwb if pi else 0)
    chunk_of = {}
    for ci, (pi, s, n) in enumerate(chunks):
        for j in range(s, s + n):
            chunk_of[(pi, j)] = ci

    from contextlib import ExitStack
    with ExitStack() as ctx:
        dsb = [ctx.enter_context(nc.sbuf_tensor(f"dsb{i}", [128, WCOL], mmdt))
               for i in range(NB)]
        tm = [ctx.enter_context(nc.sbuf_tensor(f"tm{i}", [128, CH * SLOTS], f32))
              for i in range(2)]
        ps = [ctx.enter_context(nc.psum_tensor(f"ps{i}", [128, 4 * 512], f32))
              for i in range(2)]
        s_sl = [ctx.enter_context(nc.semaphore(f"s_sl{i}")) for i in range(NB)]
        s_do = [ctx.enter_context(nc.semaphore(f"s_do{i}")) for i in range(2)]
        s_mm = ctx.enter_context(nc.semaphore("s_mm"))
        s_red = ctx.enter_context(nc.semaphore("s_red"))
        block = ctx.enter_context(nc.Block())
        dws = [dwb, dwf]
        # NOTE on DMA semaphores: completions across HWDGE queues are
        # unordered, so a semaphore value certifies a COUNT of completions,
        # not a set.  Per-parity slab semaphores + issue gating (a slab DMA
        # for wave w is only issued after its buffer's previous user
        # finished) make the count imply the exact prefix set.

        @block.sync
        def _(sync):
            # the psum-half-free gate (s_red) lives HERE, not on the tensor
            # engine: the slab DMA for wave w is only issued once the reduce
            # of wave w-2 is done, so its completion (s_sl) certifies both
            # "slab loaded" and "psum half free".  This leaves the tensor
            # engine a single wait per wave (wait-wake latency is ~0.5us).
            for w, (pi, j) in enumerate(waves):
                if w >= NB:
                    sync.wait_ge(s_mm, w - NB + 1)
                if w >= 2:
                    sync.wait_ge(s_red, w - 1)
                for g in range(4):
                    sync.dma_start(
                        dsb[w % NB][32 * g:32 * g + 5, :],
                        dws[pi][5 * g:5 * g + 5, j * WCOL:(j + 1) * WCOL],
                    ).then_inc(s_sl[w % NB], 16)

        @block.scalar
        def _(scalar):
            # out-DMAs go through the Activation engine's HWDGE queue so they
            # are not serialized behind the sync slab stream.
            tms = [tmb, tmf]
            for ci, (pi, s, n) in enumerate(chunks):
                scalar.wait_ge(s_red, gw(pi, s + n - 1) + 1)
                scalar.dma_start(
                    tms[pi][:, s * SLOTS:(s + n) * SLOTS],
                    tm[ci % 2][:, : n * SLOTS],
                ).then_inc(s_do[ci % 2], 16)

        @block.tensor
        def _(tensor):
            for w, (pi, j) in enumerate(waves):
                # s_sl completion implies the psum half is free too (the sync
                # engine gates the slab DMA issue on s_red)
                tensor.wait_ge(s_sl[w % NB], 16 * 4 * (w // NB + 1))
                for s in range(SLOTS):
                    k = s % 4
                    h = s // 4
                    mm = tensor.matmul(
                        ps[w % 2][:, k * 512 + h * T:k * 512 + h * T + T],
                        dsb[w % NB][32 * k:32 * k + 5, 4 * T + h * 128:4 * T + h * 128 + 128],
                        dsb[w % NB][32 * k:32 * k + 5, h * T:h * T + T],
                        start=(s < 4), stop=(s >= SLOTS - 4),
                        tile_position=(32 * k, 0),
                    )
                    if s == SLOTS - 1:
                        mm.then_inc(s_mm)

        @block.vector
        def _(vector):
            for w, (pi, j) in enumerate(waves):
                ci = chunk_of[(pi, j)]
                c0 = chunks[ci][1]  # first wave-in-phase of this chunk
                if j == c0 and ci >= 2:
                    vector.wait_ge(s_do[ci % 2], 16 * (ci // 2))  # tm buf free
                vector.wait_ge(s_mm, w + 1)
                src = ps[w % 2][:, : 4 * 512].rearrange(
                    "p (b t c) -> p b t c", t=HPB, c=T)
                vector.tensor_reduce(
                    tm[ci % 2][:, (j - c0) * SLOTS:(j - c0 + 1) * SLOTS], src,
                    axis=mybir.AxisListType.X, op=mybir.AluOpType.min,
                ).then_inc(s_red)
    return nc


# psum/out column order of slot s within a wave: bank (s%4), sub-tile (s//4),
# i.e. reduce output column = (s%4)*HPB + (s//4)
OUTCOL = [(s % 4) * HPB + (s // 4) for s in range(SLOTS)]


# ------------------------------ host side ----------------------------------

def _layout3d(q, grid, R):
    """Equal-count 3D binning of queries into blocks of 128.

    Returns qperm (block-major query order) and per-block inflated boxes
    (lo [nb,3], hi [nb,3]).  Guarantee: any db point outside a block's box
    differs > R in x, y or z from every query of the block."""
    nx, ny, nz = grid
    nq = q.shape[0]
    per_x = nq // nx
    per_y = per_x // ny
    per_z = per_y // nz
    assert per_z == 128
    xo = np.argsort(q[:, 0], kind="stable")
    qperm = np.empty(nq, np.int64)
    pos = 0
    for ix in range(nx):
        qi = xo[ix * per_x:(ix + 1) * per_x]
        yo = qi[np.argsort(q[qi, 1], kind="stable")]
        for iy in range(ny):
            qj = yo[iy * per_y:(iy + 1) * per_y]
            zo = qj[np.argsort(q[qj, 2], kind="stable")]
            qperm[pos:pos + per_y] = zo
            pos += per_y
    nb = nq // 128
    qs = q[qperm].reshape(nb, 128, 3)
    lo = qs.min(axis=1) - R
    hi = qs.max(axis=1) + R
    return qperm, lo, hi


def _block_candidates(db, lo, hi):
    """Exact per-block box membership via x-sorted slicing.

    Returns list of index arrays (positions into ORIGINAL db)."""
    xo = np.argsort(db[:, 0], kind="stable")
    dbx = db[xo]
    out = []
    for b in range(lo.shape[0]):
        a0 = np.searchsorted(dbx[:, 0], lo[b, 0], side="left")
        a1 = np.searchsorted(dbx[:, 0], hi[b, 0], side="right")
        seg = dbx[a0:a1]
        m = ((seg[:, 1] >= lo[b, 1]) & (seg[:, 1] <= hi[b, 1])
             & (seg[:, 2] >= lo[b, 2]) & (seg[:, 2] <= hi[b, 2]))
        idx = xo[a0:a1][m]
        if idx.size == 0:
            idx = np.zeros(1, np.int64)
        out.append(idx)
    return out


def _pack_core(cands, df, qf, b0, b1, nw):
    """Pack blocks [b0,b1) into the wave/slot stream of one core.

    cands: per-block candidate index lists (original db positions).
    df [5, Ndb]: db features.  qf [5, nbtot*128]: query features (block-major,
    global block ids).  Returns (dwarr [20, nw*WCOL], slot_blocks [nw*8]
    block-in-core ids, gidx [b1-b0, ntmax*T], nts, ntmax)."""
    slots = []           # (block-in-core, tile positions [T])
    gidx_rows = []
    nts = []
    for bi, b in enumerate(range(b0, b1)):
        idx = cands[b]
        nt = -(-len(idx) // T)
        reps = -(-nt * T // len(idx))
        padded = np.tile(idx, reps)[: nt * T]
        gidx_rows.append(padded)
        nts.append(nt)
        for t in range(nt):
            slots.append((bi, padded[t * T:(t + 1) * T]))
    assert len(slots) <= nw * SLOTS, (len(slots), nw)
    while len(slots) < nw * SLOTS:
        slots.append(slots[0])  # dummy padding (ignored by host)
    slot_blocks = np.array([s[0] for s in slots], np.int64)
    dwarr = np.empty((20, nw * WCOL), np.float32)
    for w in range(nw):
        for s in range(SLOTS):
            bi, pos = slots[w * SLOTS + s]
            g = s % 4
            h = s // 4
            base = w * WCOL
            dwarr[5 * g:5 * g + 5, base + h * T:base + h * T + T] = df[:, pos]
            dwarr[5 * g:5 * g + 5,
                  base + 4 * T + h * 128:base + 4 * T + h * 128 + 128] = \
                qf[:, (b0 + bi) * 128:(b0 + bi + 1) * 128]
    ntmax = max(nts)
    gidx = np.zeros((b1 - b0, ntmax * T), np.int64)
    for bi, row in enumerate(gidx_rows):
        gidx[bi, : len(row)] = row
    return dwarr, slot_blocks, gidx, np.array(nts), ntmax


def _row_screen(tmin_rows, gidx, q, qq, db_s, pp_s, delta):
    """Vectorized: exact (clamped min, ties) per row from device screen.

    tmin_rows [R, NT] device tile minima (+inf for missing tiles),
    gidx [R//128, NT*T] candidate positions.  Returns mins [R] (clamped),
    ties list of positions, needs_fallback bool [R]."""
    Rn, NT = tmin_rows.shape
    m_dev = tmin_rows.min(axis=1)
    order = np.argsort(tmin_rows, axis=1)
    mins = np.full(Rn, np.inf, np.float32)
    active = np.ones(Rn, bool)
    rank = 0
    CHR = 8192
    pos_all = [[] for _ in range(Rn)]
    while active.any() and rank < NT:
        rows = np.nonzero(active)[0]
        tiles = order[rows, rank]
        in_win = tmin_rows[rows, tiles] <= m_dev[rows] + delta
        rows = rows[in_win]
        tiles = tiles[in_win]
        active[:] = False
        active[rows] = True
        for s in range(0, len(rows), CHR):
            r = rows[s:s + CHR]
            tl = tiles[s:s + CHR]
            cand = gidx[r // 128][np.arange(len(r))[:, None],
                                  tl.astype(np.int64)[:, None] * T + np.arange(T)[None, :]]
            vals = d2_rows(q[r], qq[r], db_s[cand], pp_s[cand])
            np.maximum(vals, 0.0, out=vals)
            vmin = vals.min(axis=1)
            upd = vmin < mins[r]
            eq = vmin == mins[r]
            mins[r] = np.minimum(mins[r], vmin)
            tie_rows, tie_cols = np.nonzero(vals == mins[r][:, None])
            bounds = np.searchsorted(tie_rows, np.arange(len(r) + 1))
            for k in np.nonzero(upd | eq)[0]:
                sel = tie_cols[bounds[k]:bounds[k + 1]]
                if sel.size == 0:
                    continue
                p = cand[k, sel].tolist()
                ri = int(r[k])
                if upd[k]:
                    pos_all[ri] = p
                else:
                    pos_all[ri].extend(p)
        rank += 1
    needs_fb = mins > m_dev + EPS_CERT
    return mins, pos_all, needs_fb


def _kd_fallback(rows, q, qq, db_s, pp_s, tree):
    """Exact (reference-arithmetic) NN + ties for fallback rows via KDTree.

    The tree gives the geometric NN distance; all points whose geometric d2
    is within +1.0 of it are recomputed bit-exactly (reference fp32 d2 and
    geometric d2 differ by < ~0.2 at these magnitudes, so the ball covers
    every point that could be or tie the reference minimum)."""
    qr = q[rows]
    d, _ = tree.query(qr, k=1, workers=-1)
    r = np.sqrt(d * d + 1.0)
    balls = tree.query_ball_point(qr, r, workers=-1)
    mins = np.empty(len(rows), np.float32)
    out_pos = []
    for k in range(len(rows)):
        idx = np.asarray(balls[k], np.int64)
        vals = d2_rows(qr[k:k + 1], qq[rows[k]:rows[k] + 1],
                       db_s[idx][None], pp_s[idx][None])[0]
        np.maximum(vals, 0.0, out=vals)
        mn = vals.min()
        mins[k] = mn
        out_pos.append(idx[vals == mn].tolist())
    return mins, out_pos


def _knn_pass(q_orig, db_orig, tmin, gidx, qperm, radius2):
    """Assemble exact per-ORIGINAL-row (min, tie orig-idx list) for one pass.

    tmin [Rn, NTmax] device tile minima (inf-padded); gidx positions are
    ORIGINAL db indices."""
    q_s = q_orig[qperm]
    qq_s = sq_query(q_orig)[qperm]
    pp = sq_db(db_orig)
    Rn = tmin.shape[0]

    mins, pos, needs_fb = _row_screen(tmin, gidx, q_s, qq_s, db_orig, pp, DELTA)
    fb = np.nonzero(needs_fb | (mins > radius2 - 1.0))[0]
    if len(fb):
        from scipy.spatial import cKDTree
        tree = cKDTree(db_orig)
        fmins, fpos = _kd_fallback(fb, q_s, qq_s, db_orig, pp, tree)
        for k, r in enumerate(fb):
            mins[r] = fmins[k]
            pos[r] = fpos[k]
    mins_o = np.empty_like(mins)
    ties_o = [None] * Rn
    for r in range(Rn):
        mins_o[qperm[r]] = mins[r]
        ties_o[qperm[r]] = np.unique(np.asarray(pos[r], np.int64))
    return mins_o, ties_o


def _finish(pred, tgt, rgb, bmin, bties, fmin, fties):
    """Reference epilogue, bit-faithful (np.add.at == XLA scatter-add order)."""
    accum = np.zeros((M, 3), np.float32)
    denom = np.zeros(M, np.float32)
    EPS = np.float32(1e-30)
    w_all = (np.float64(1.0) /
             np.sqrt(np.maximum(bmin, EPS).astype(np.float64))).astype(np.float32)
    nz = bmin > 0.0
    counts = np.array([len(bties[n]) if nz[n] else 0 for n in range(N)], np.int64)
    row_a = np.repeat(np.arange(N), counts)
    idx_a = np.concatenate([bties[n] for n in range(N) if nz[n] and len(bties[n])]
                           ) if counts.sum() else np.zeros(0, np.int64)
    w_a = w_all[row_a]
    np.add.at(accum, idx_a, (w_a[:, None] * rgb[row_a]).astype(np.float32))
    np.add.at(denom, idx_a, w_a)
    has_w = denom != 0.0
    recolored = np.where(
        has_w[:, None],
        (accum / np.where(has_w, denom, np.float32(1.0))[:, None]).astype(np.float32),
        np.float32(0.0)).astype(np.float32)
    zero_assigned = np.zeros(M, bool)
    for n in np.nonzero(bmin == 0.0)[0]:
        for j in bties[n]:
            recolored[j] = rgb[n]
            zero_assigned[j] = True
    empty = (~has_w) & (~zero_assigned)
    out = recolored
    for i in np.nonzero(empty)[0]:
        t = fties[i]
        s = np.zeros(3, np.float32)
        for j in t:
            s = (s + rgb[j]).astype(np.float32)
        out[i] = (s / np.float32(len(t))).astype(np.float32)
    return out


def _install_ntff_hook():
    """Provide antenv.axon_hooks (absent on some images) and register the
    ctypes NTFF profile hook so run_bass_kernel_spmd(trace=True) works."""
    import types
    try:
        from antenv.axon_hooks import get_axon_ntff_profile_hook  # noqa: F401
        import antenv.axon_hooks as hooks_mod
    except ImportError:
        try:
            import antenv
        except ImportError:
            return
        hooks_mod = types.ModuleType("antenv.axon_hooks")
        hooks_mod._hook = None

        def _set(h):
            hooks_mod._hook = h

        def _get():
            return hooks_mod._hook

        hooks_mod.set_axon_ntff_profile_hook = _set
        hooks_mod.get_axon_ntff_profile_hook = _get
        sys.modules["antenv.axon_hooks"] = hooks_mod
        antenv.axon_hooks = hooks_mod
    if hooks_mod.get_axon_ntff_profile_hook() is None:
        try:
            from trn_agent_boot.trn_boot import _ntff_profile_via_ctypes
            hook = _ntff_profile_via_ctypes("/opt/axon/libaxon_pjrt.so")
            if hook is not None:
                hooks_mod.set_axon_ntff_profile_hook(hook)
        except Exception:
            pass


def _prep_pass(q, db, grid, nb_core):
    """Layout + candidate gather + per-core packing for one KNN pass."""
    qperm, lo, hi = _layout3d(q, grid, RADIUS)
    cands = _block_candidates(db, lo, hi)
    nslots = [sum(-(-len(cands[b]) // T) for b in range(c * nb_core, (c + 1) * nb_core))
              for c in range(CORES)]
    nw = -(-max(nslots) // SLOTS)

    qf = np.ascontiguousarray(np.stack([
        sq_query(q)[qperm].astype(np.float32),
        np.ones(len(qperm), np.float32),
        _f32(-2.0 * q[qperm, 0]), _f32(-2.0 * q[qperm, 1]), _f32(-2.0 * q[qperm, 2]),
    ]).astype(np.float32))
    df = np.ascontiguousarray(np.stack([
        np.ones(db.shape[0], np.float32), sq_db(db).astype(np.float32),
        db[:, 0], db[:, 1], db[:, 2]]).astype(np.float32))

    cores = []
    gidx_all = []
    nts_all = []
    ntmax = 0
    for c in range(CORES):
        dwarr, slot_blocks, gidx, nts, ntm = _pack_core(
            cands, df, qf, c * nb_core, (c + 1) * nb_core, nw)
        cores.append((dwarr, slot_blocks))
        gidx_all.append(gidx)
        nts_all.append(nts)
        ntmax = max(ntmax, ntm)
    nblocks = len(cands)
    gidx_full = np.zeros((nblocks, ntmax * T), np.int64)
    for c in range(CORES):
        g = gidx_all[c]
        gidx_full[c * nb_core:(c + 1) * nb_core, : g.shape[1]] = g
    nts_full = np.concatenate(nts_all)
    return dict(qperm=qperm, cores=cores, gidx=gidx_full, nts=nts_full,
                ntmax=ntmax, nw=nw)


def _assemble_tmin(tm_cores, slot_blocks_cores, nts, nb_core, ntmax):
    """Map device per-slot minima back to [Rn, NTmax] (inf-padded)."""
    nblocks = len(nts)
    out = np.full((nblocks * 128, ntmax), np.inf, np.float32)
    for c, tmo in enumerate(tm_cores):
        nw = tmo.shape[1] // SLOTS
        sb = slot_blocks_cores[c]
        tile_seen = {}
        for i in range(nw * SLOTS):
            bi = int(sb[i])
            t = tile_seen.get(bi, 0)
            b = c * nb_core + bi
            if t < nts[b]:
                w, s = divmod(i, SLOTS)
                out[b * 128:(b + 1) * 128, t] = tmo[:, w * SLOTS + OUTCOL[s]]
                tile_seen[bi] = t + 1
    return out


def kernel(pred_xyz, tgt_xyz, tgt_rgb, search_range):
    global _LAST_RESULTS
    from concourse.bass_utils import run_bass_kernel_spmd

    pred = np.ascontiguousarray(np.asarray(pred_xyz, dtype=np.float32))
    tgt = np.ascontiguousarray(np.asarray(tgt_xyz, dtype=np.float32))
    rgb = np.ascontiguousarray(np.asarray(tgt_rgb, dtype=np.float32))
    assert pred.shape == (M, 3) and tgt.shape == (N, 3)

    nbb = (N // 128) // CORES   # 48 backward blocks per core
    nbf = (M // 128) // CORES   # 64 forward blocks per core

    B = _prep_pass(tgt, pred, GRID_B, nbb)    # backward: queries tgt, db pred
    F = _prep_pass(pred, tgt, GRID_F, nbf)    # forward: queries pred, db tgt

    in_maps = []
    for c in range(CORES):
        in_maps.append({
            "dwb": B["cores"][c][0],
            "dwf": F["cores"][c][0],
        })

    nc = _build_program(B["nw"], F["nw"])
    trace = bool(int(os.environ.get("KNN_TRACE", "0")))
    if trace:
        _install_ntff_hook()
    try:
        res = run_bass_kernel_spmd(nc, in_maps, core_ids=list(range(CORES)), trace=trace)
    except Exception:
        if not trace:
            raise
        res = run_bass_kernel_spmd(nc, in_maps, core_ids=list(range(CORES)), trace=False)
    _LAST_RESULTS = res

    tmin_b = _assemble_tmin([res.results[c]["tmb"] for c in range(CORES)],
                            [B["cores"][c][1] for c in range(CORES)],
                            B["nts"], nbb, B["ntmax"])
    tmin_f = _assemble_tmin([res.results[c]["tmf"] for c in range(CORES)],
                            [F["cores"][c][1] for c in range(CORES)],
                            F["nts"], nbf, F["ntmax"])

    bmin, bties = _knn_pass(tgt, pred, tmin_b, B["gidx"], B["qperm"], RADIUS * RADIUS)
    fmin, fties = _knn_pass(pred, tgt, tmin_f, F["gidx"], F["qperm"], RADIUS * RADIUS)

    return _finish(pred, tgt, rgb, bmin, bties, fmin, fties)


# revision 26
# speedup vs baseline: 1.4510x; 1.4510x over previous
"""Trainium2 8-core kernel for sample_wise_recolor (retrieval KNN).

Strategy
--------
Both KNN passes (tgt->pred "backward", pred->tgt "forward") are 1-NN-with-ties
problems: the reference only consumes the entries of the top-k that equal the
row minimum.  Queries are binned into equal-count 3D cells of 128 points
(a "block"); each block only scans the db points inside its bounding box
inflated by a safety radius R (any point outside differs > R in x, y or z,
hence d2 > R^2, which is checked and falls back to a full scan).

Device (8 NeuronCores, SPMD, blocks sharded contiguously): per-block
candidate slabs are cut into 256-wide tiles and packed into uniform
8-slot "waves" (one PSUM half = 4 banks = 8 x 256 fp32).  Each wave's DMA
carries both the db tile features and the owning block's query features
(so the slot->block mapping lives in the data, keeping the program
SPMD-identical across cores).  Per wave:
  PE  : 8 matmuls d2 = [qq,1,-2qx,-2qy,-2qz] . [1,pp,px,py,pz]  (K=5 fp32),
        4-band tile_position layout so up to 4 tiles stream concurrently
  DVE : one tensor_reduce min over the 2048-col half -> per-slot row minima
PSUM halves / slab buffers / tm buffers are multi-buffered so DMA, PE and
DVE pipeline across waves.

Host: for each row, the winning tile(s) (within DELTA of the device row-min)
are recomputed *bit-exactly* in the reference's own fp32 rounding (the XLA-CPU
recipe was reverse-engineered and verified: query squares fma(z,z,fma(x,x,
fl(y^2))), db squares fl(fl(x^2+y^2)+z^2), dot = fma chain, combine
fl(fl(qq+pp)-2B)).  This yields the exact row minimum and exact tie set, from
which the reference's scatter/divide/fallback epilogue is reproduced.
"""

import os
import sys

for _p in ("/opt/trn_rl_repo", "/root/.axon_site/_ro/trn_rl_repo"):
    if os.path.isdir(_p) and _p not in sys.path:
        sys.path.insert(0, _p)

import numpy as np

M = 65536          # pred points
N = 49152          # tgt points
T = 128            # screen tile width (quarter of a fp32 PSUM bank)
SLOTS = 16         # tiles per wave (= one PSUM half = 4 banks)
HPB = 4            # tiles per bank (bank = 512 fp32 = HPB*T)
WCOL = 4 * T + 4 * 128   # per-band DRAM cols per wave: 4 db tiles + 4 query sets
CORES = 8
NB = 6             # slab DMA buffers
CH = 16            # waves per tm out-DMA chunk
GRID_B = (8, 8, 6)   # backward pass query grid (tgt, 384 blocks)
GRID_F = (8, 8, 8)   # forward pass query grid (pred, 512 blocks)
RADIUS = 6.0       # slab safety radius
DELTA = 0.25       # screen window width (>> device matmul error ~0.03)
EPS_CERT = 0.12    # asserted bound on |device tile min - exact tile min|
SPLITRED = bool(int(os.environ.get("KNN_SPLITRED", "1")))  # 2 reduce ops/wave

_LAST_RESULTS = None  # BassKernelResults of the last device run (for test.py)


def _f32(a):
    return np.asarray(a, dtype=np.float32)


# ----- bit-exact XLA-CPU fp32 arithmetic emulation (verified vs reference) --

def sq_query(a):
    """jnp.sum(qc*qc, axis=1) inside the per-chunk jit: fma(z,z, fma(x,x, fl(y*y)))."""
    x, z = a[..., 0].astype(np.float64), a[..., 2].astype(np.float64)
    s = _f32(a[..., 1] * a[..., 1]).astype(np.float64)
    s = _f32(x * x + s).astype(np.float64)
    return _f32(z * z + s)


def sq_db(a):
    """jnp.sum(db*db, axis=1) standalone kernel: fl(fl(x^2+y^2)+z^2)."""
    return _f32(_f32(_f32(a[..., 0] * a[..., 0]) + _f32(a[..., 1] * a[..., 1]))
                + _f32(a[..., 2] * a[..., 2]))


def d2_rows(q, qq, db3, pp3):
    """Bit-exact pre-clamp d2 for per-row candidate sets.

    q [R,3], qq [R] (sq_query), db3 [R,C,3], pp3 [R,C] (sq_db gathered)."""
    qb = q[:, None, :]
    r = _f32(qb[..., 0] * db3[..., 0]).astype(np.float64)
    r = _f32(qb[..., 1].astype(np.float64) * db3[..., 1].astype(np.float64) + r).astype(np.float64)
    B = _f32(qb[..., 2].astype(np.float64) * db3[..., 2].astype(np.float64) + r)
    return _f32(_f32(qq[:, None] + pp3) - _f32(2.0) * B)


# --------------------------- device program --------------------------------

def _build_program(nwb, nwf):
    import concourse.bass as bass
    import concourse.mybir as mybir

    nc = bass.Bass()
    f32 = mybir.dt.float32
    # matmul input dtype: float32 (2-pass, exact) or float32r (1-pass, lower
    # precision; byte-compatible with fp32 so only the tensor dtype changes)
    mmdt = {"f32": mybir.dt.float32, "f32r": mybir.dt.float32r}[
        os.environ.get("KNN_MMDT", "f32")]
    # per-wave combined slabs, band-grouped: rows 5g:5g+5, cols w*WCOL +
    # [0:T]=db slot g, [T:2T]=db slot g+4, [2T:2T+128]=q slot g,
    # [2T+128:2T+256]=q slot g+4
    dwb = nc.dram_tensor("dwb", [5 * 4, nwb * WCOL], mmdt, kind="ExternalInput")
    dwf = nc.dram_tensor("dwf", [5 * 4, nwf * WCOL], mmdt, kind="ExternalInput")
    tmb = nc.dram_tensor("tmb", [128, nwb * SLOTS], f32, kind="ExternalOutput")
    tmf = nc.dram_tensor("tmf", [128, nwf * SLOTS], f32, kind="ExternalOutput")

    waves = [(0, j) for j in range(nwb)] + [(1, j) for j in range(nwf)]
    chunks = []
    for pi, nw in ((0, nwb), (1, nwf)):
        for s in range(0, nw, CH):
            chunks.append((pi, s, min(CH, nw - s)))
    gw = lambda pi, j: j + (nwb if pi else 0)
    chunk_of = {}
    for ci, (pi, s, n) in enumerate(chunks):
        for j in range(s, s + n):
            chunk_of[(pi, j)] = ci

    from contextlib import ExitStack
    with ExitStack() as ctx:
        dsb = [ctx.enter_context(nc.sbuf_tensor(f"dsb{i}", [128, WCOL], mmdt))
               for i in range(NB)]
        tm = [ctx.enter_context(nc.sbuf_tensor(f"tm{i}", [128, CH * SLOTS], f32))
              for i in range(2)]
        ps = [ctx.enter_context(nc.psum_tensor(f"ps{i}", [128, 4 * 512], f32))
              for i in range(2)]
        s_sl = [ctx.enter_context(nc.semaphore(f"s_sl{i}")) for i in range(NB)]
        s_do = [ctx.enter_context(nc.semaphore(f"s_do{i}")) for i in range(2)]
        s_mm = ctx.enter_context(nc.semaphore("s_mm"))
        s_red = ctx.enter_context(nc.semaphore("s_red"))
        block = ctx.enter_context(nc.Block())
        dws = [dwb, dwf]
        # NOTE on DMA semaphores: completions across HWDGE queues are
        # unordered, so a semaphore value certifies a COUNT of completions,
        # not a set.  Per-parity slab semaphores + issue gating (a slab DMA
        # for wave w is only issued after its buffer's previous user
        # finished) make the count imply the exact prefix set.

        @block.sync
        def _(sync):
            for w, (pi, j) in enumerate(waves):
                if w >= NB:
                    sync.wait_ge(s_mm, 2 * (w - NB) + 2)
                for g in range(4):
                    sync.dma_start(
                        dsb[w % NB][32 * g:32 * g + 5, :],
                        dws[pi][5 * g:5 * g + 5, j * WCOL:(j + 1) * WCOL],
                    ).then_inc(s_sl[w % NB], 16)

        @block.scalar
        def _(scalar):
            # out-DMAs go through the Activation engine's HWDGE queue so they
            # are not serialized behind the sync slab stream.
            tms = [tmb, tmf]
            for ci, (pi, s, n) in enumerate(chunks):
                scalar.wait_ge(s_red, 2 * (gw(pi, s + n - 1) + 1))
                scalar.dma_start(
                    tms[pi][:, s * SLOTS:(s + n) * SLOTS],
                    tm[ci % 2][:, : n * SLOTS],
                ).then_inc(s_do[ci % 2], 16)

        @block.tensor
        def _(tensor):
            # the wave's reduce is split in two (psum cols 0:2T and 2T:4T of
            # each bank), so the PE regains each region as soon as its half-
            # reduce of wave w-2 lands, and the first half-reduce of wave w
            # overlaps the PE's second half.
            for w, (pi, j) in enumerate(waves):
                tensor.wait_ge(s_sl[w % NB], 16 * 4 * (w // NB + 1))
                for s in range(SLOTS):
                    k = s % 4
                    h = s // 4
                    if w >= 2 and s in (0, SLOTS // 2):
                        tensor.wait_ge(s_red, 2 * (w - 2) + (1 if s == 0 else 2))
                    mm = tensor.matmul(
                        ps[w % 2][:, k * 512 + h * T:k * 512 + h * T + T],
                        dsb[w % NB][32 * k:32 * k + 5, 4 * T + h * 128:4 * T + h * 128 + 128],
                        dsb[w % NB][32 * k:32 * k + 5, h * T:h * T + T],
                        start=(s < 4), stop=(s >= SLOTS - 4),
                        tile_position=(32 * k, 0),
                    )
                    if s in (SLOTS // 2 - 1, SLOTS - 1):
                        mm.then_inc(s_mm)

        @block.vector
        def _(vector):
            for w, (pi, j) in enumerate(waves):
                ci = chunk_of[(pi, j)]
                c0 = chunks[ci][1]  # first wave-in-phase of this chunk
                if j == c0 and ci >= 2:
                    vector.wait_ge(s_do[ci % 2], 16 * (ci // 2))  # tm buf free
                base = (j - c0) * SLOTS
                if SPLITRED:
                    for half in range(2):
                        vector.wait_ge(s_mm, 2 * w + half + 1)
                        src = ps[w % 2][:, : 4 * 512].rearrange(
                            "p (b x) -> p b x", x=512)[
                            :, :, half * 2 * T:(half + 1) * 2 * T].rearrange(
                            "p b (t c) -> p b t c", c=T)
                        vector.tensor_reduce(
                            tm[ci % 2][:, base + half * 8:base + half * 8 + 8], src,
                            axis=mybir.AxisListType.X, op=mybir.AluOpType.min,
                        ).then_inc(s_red)
                else:
                    vector.wait_ge(s_mm, 2 * w + 2)
                    src = ps[w % 2][:, : 4 * 512].rearrange(
                        "p (b t c) -> p b t c", t=HPB, c=T)
                    vector.tensor_reduce(
                        tm[ci % 2][:, base:base + SLOTS], src,
                        axis=mybir.AxisListType.X, op=mybir.AluOpType.min,
                    ).then_inc(s_red, 2)
    return nc


# psum/out column order of slot s within a wave (bank k=s%4, sub-tile h=s//4):
# split mode: two reduce ops, each writing 8 contiguous cols in (bank, t) order
# single mode: one reduce op in (bank, sub-tile) order
if SPLITRED:
    OUTCOL = [(s // 8) * 8 + (s % 4) * 2 + ((s // 4) % 2) for s in range(SLOTS)]
else:
    OUTCOL = [(s % 4) * HPB + (s // 4) for s in range(SLOTS)]


# ------------------------------ host side ----------------------------------

def _layout3d(q, grid, R):
    """Equal-count 3D binning of queries into blocks of 128.

    Returns qperm (block-major query order) and per-block inflated boxes
    (lo [nb,3], hi [nb,3]).  Guarantee: any db point outside a block's box
    differs > R in x, y or z from every query of the block."""
    nx, ny, nz = grid
    nq = q.shape[0]
    per_x = nq // nx
    per_y = per_x // ny
    per_z = per_y // nz
    assert per_z == 128
    xo = np.argsort(q[:, 0], kind="stable")
    qperm = np.empty(nq, np.int64)
    pos = 0
    for ix in range(nx):
        qi = xo[ix * per_x:(ix + 1) * per_x]
        yo = qi[np.argsort(q[qi, 1], kind="stable")]
        for iy in range(ny):
            qj = yo[iy * per_y:(iy + 1) * per_y]
            zo = qj[np.argsort(q[qj, 2], kind="stable")]
            qperm[pos:pos + per_y] = zo
            pos += per_y
    nb = nq // 128
    qs = q[qperm].reshape(nb, 128, 3)
    lo = qs.min(axis=1) - R
    hi = qs.max(axis=1) + R
    return qperm, lo, hi


def _block_candidates(db, lo, hi):
    """Exact per-block box membership via x-sorted slicing.

    Returns list of index arrays (positions into ORIGINAL db)."""
    xo = np.argsort(db[:, 0], kind="stable")
    dbx = db[xo]
    out = []
    for b in range(lo.shape[0]):
        a0 = np.searchsorted(dbx[:, 0], lo[b, 0], side="left")
        a1 = np.searchsorted(dbx[:, 0], hi[b, 0], side="right")
        seg = dbx[a0:a1]
        m = ((seg[:, 1] >= lo[b, 1]) & (seg[:, 1] <= hi[b, 1])
             & (seg[:, 2] >= lo[b, 2]) & (seg[:, 2] <= hi[b, 2]))
        idx = xo[a0:a1][m]
        if idx.size == 0:
            idx = np.zeros(1, np.int64)
        out.append(idx)
    return out


def _pack_core(cands, df, qf, blist, nw):
    """Pack the blocks of one core (global ids `blist`) into its wave/slot
    stream.

    cands: per-block candidate index lists (original db positions).
    df [5, Ndb]: db features.  qf [5, nbtot*128]: query features (block-major,
    global block ids).  Returns (dwarr [20, nw*WCOL], slot_blocks [nw*SLOTS]
    block-in-core ids, gidx_rows per block, nts)."""
    slots = []           # (block-in-core, tile positions [T])
    gidx_rows = []
    nts = []
    for bi, b in enumerate(blist):
        idx = cands[b]
        nt = -(-len(idx) // T)
        reps = -(-nt * T // len(idx))
        padded = np.tile(idx, reps)[: nt * T]
        gidx_rows.append(padded)
        nts.append(nt)
        for t in range(nt):
            slots.append((bi, padded[t * T:(t + 1) * T]))
    assert len(slots) <= nw * SLOTS, (len(slots), nw)
    while len(slots) < nw * SLOTS:
        slots.append(slots[0])  # dummy padding (ignored by host)
    slot_blocks = np.array([s[0] for s in slots], np.int64)
    dwarr = np.empty((20, nw * WCOL), np.float32)
    for w in range(nw):
        for s in range(SLOTS):
            bi, pos = slots[w * SLOTS + s]
            g = s % 4
            h = s // 4
            base = w * WCOL
            dwarr[5 * g:5 * g + 5, base + h * T:base + h * T + T] = df[:, pos]
            b = blist[bi]
            dwarr[5 * g:5 * g + 5,
                  base + 4 * T + h * 128:base + 4 * T + h * 128 + 128] = \
                qf[:, b * 128:(b + 1) * 128]
    return dwarr, slot_blocks, gidx_rows, nts


def _row_screen(tmin_rows, gidx, q, qq, db_s, pp_s, delta):
    """Vectorized: exact (clamped min, ties) per row from device screen.

    tmin_rows [R, NT] device tile minima (+inf for missing tiles),
    gidx [R//128, NT*T] candidate positions.  Returns mins [R] (clamped),
    ties list of positions, needs_fallback bool [R]."""
    Rn, NT = tmin_rows.shape
    m_dev = tmin_rows.min(axis=1)
    order = np.argsort(tmin_rows, axis=1)
    mins = np.full(Rn, np.inf, np.float32)
    active = np.ones(Rn, bool)
    rank = 0
    CHR = 8192
    pos_all = [[] for _ in range(Rn)]
    while active.any() and rank < NT:
        rows = np.nonzero(active)[0]
        tiles = order[rows, rank]
        in_win = tmin_rows[rows, tiles] <= m_dev[rows] + delta
        rows = rows[in_win]
        tiles = tiles[in_win]
        active[:] = False
        active[rows] = True
        for s in range(0, len(rows), CHR):
            r = rows[s:s + CHR]
            tl = tiles[s:s + CHR]
            cand = gidx[r // 128][np.arange(len(r))[:, None],
                                  tl.astype(np.int64)[:, None] * T + np.arange(T)[None, :]]
            vals = d2_rows(q[r], qq[r], db_s[cand], pp_s[cand])
            np.maximum(vals, 0.0, out=vals)
            vmin = vals.min(axis=1)
            upd = vmin < mins[r]
            eq = vmin == mins[r]
            mins[r] = np.minimum(mins[r], vmin)
            tie_rows, tie_cols = np.nonzero(vals == mins[r][:, None])
            bounds = np.searchsorted(tie_rows, np.arange(len(r) + 1))
            for k in np.nonzero(upd | eq)[0]:
                sel = tie_cols[bounds[k]:bounds[k + 1]]
                if sel.size == 0:
                    continue
                p = cand[k, sel].tolist()
                ri = int(r[k])
                if upd[k]:
                    pos_all[ri] = p
                else:
                    pos_all[ri].extend(p)
        rank += 1
    needs_fb = mins > m_dev + EPS_CERT
    return mins, pos_all, needs_fb


def _kd_fallback(rows, q, qq, db_s, pp_s, tree):
    """Exact (reference-arithmetic) NN + ties for fallback rows via KDTree.

    The tree gives the geometric NN distance; all points whose geometric d2
    is within +1.0 of it are recomputed bit-exactly (reference fp32 d2 and
    geometric d2 differ by < ~0.2 at these magnitudes, so the ball covers
    every point that could be or tie the reference minimum)."""
    qr = q[rows]
    d, _ = tree.query(qr, k=1, workers=-1)
    r = np.sqrt(d * d + 1.0)
    balls = tree.query_ball_point(qr, r, workers=-1)
    mins = np.empty(len(rows), np.float32)
    out_pos = []
    for k in range(len(rows)):
        idx = np.asarray(balls[k], np.int64)
        vals = d2_rows(qr[k:k + 1], qq[rows[k]:rows[k] + 1],
                       db_s[idx][None], pp_s[idx][None])[0]
        np.maximum(vals, 0.0, out=vals)
        mn = vals.min()
        mins[k] = mn
        out_pos.append(idx[vals == mn].tolist())
    return mins, out_pos


def _knn_pass(q_orig, db_orig, tmin, gidx, qperm, radius2):
    """Assemble exact per-ORIGINAL-row (min, tie orig-idx list) for one pass.

    tmin [Rn, NTmax] device tile minima (inf-padded); gidx positions are
    ORIGINAL db indices."""
    q_s = q_orig[qperm]
    qq_s = sq_query(q_orig)[qperm]
    pp = sq_db(db_orig)
    Rn = tmin.shape[0]

    mins, pos, needs_fb = _row_screen(tmin, gidx, q_s, qq_s, db_orig, pp, DELTA)
    fb = np.nonzero(needs_fb | (mins > radius2 - 1.0))[0]
    if len(fb):
        from scipy.spatial import cKDTree
        tree = cKDTree(db_orig)
        fmins, fpos = _kd_fallback(fb, q_s, qq_s, db_orig, pp, tree)
        for k, r in enumerate(fb):
            mins[r] = fmins[k]
            pos[r] = fpos[k]
    mins_o = np.empty_like(mins)
    ties_o = [None] * Rn
    for r in range(Rn):
        mins_o[qperm[r]] = mins[r]
        ties_o[qperm[r]] = np.unique(np.asarray(pos[r], np.int64))
    return mins_o, ties_o


def _finish(pred, tgt, rgb, bmin, bties, fmin, fties):
    """Reference epilogue, bit-faithful (np.add.at == XLA scatter-add order)."""
    accum = np.zeros((M, 3), np.float32)
    denom = np.zeros(M, np.float32)
    EPS = np.float32(1e-30)
    w_all = (np.float64(1.0) /
             np.sqrt(np.maximum(bmin, EPS).astype(np.float64))).astype(np.float32)
    nz = bmin > 0.0
    counts = np.array([len(bties[n]) if nz[n] else 0 for n in range(N)], np.int64)
    row_a = np.repeat(np.arange(N), counts)
    idx_a = np.concatenate([bties[n] for n in range(N) if nz[n] and len(bties[n])]
                           ) if counts.sum() else np.zeros(0, np.int64)
    w_a = w_all[row_a]
    np.add.at(accum, idx_a, (w_a[:, None] * rgb[row_a]).astype(np.float32))
    np.add.at(denom, idx_a, w_a)
    has_w = denom != 0.0
    recolored = np.where(
        has_w[:, None],
        (accum / np.where(has_w, denom, np.float32(1.0))[:, None]).astype(np.float32),
        np.float32(0.0)).astype(np.float32)
    zero_assigned = np.zeros(M, bool)
    for n in np.nonzero(bmin == 0.0)[0]:
        for j in bties[n]:
            recolored[j] = rgb[n]
            zero_assigned[j] = True
    empty = (~has_w) & (~zero_assigned)
    out = recolored
    for i in np.nonzero(empty)[0]:
        t = fties[i]
        s = np.zeros(3, np.float32)
        for j in t:
            s = (s + rgb[j]).astype(np.float32)
        out[i] = (s / np.float32(len(t))).astype(np.float32)
    return out


def _install_ntff_hook():
    """Provide antenv.axon_hooks (absent on some images) and register the
    ctypes NTFF profile hook so run_bass_kernel_spmd(trace=True) works."""
    import types
    try:
        from antenv.axon_hooks import get_axon_ntff_profile_hook  # noqa: F401
        import antenv.axon_hooks as hooks_mod
    except ImportError:
        try:
            import antenv
        except ImportError:
            return
        hooks_mod = types.ModuleType("antenv.axon_hooks")
        hooks_mod._hook = None

        def _set(h):
            hooks_mod._hook = h

        def _get():
            return hooks_mod._hook

        hooks_mod.set_axon_ntff_profile_hook = _set
        hooks_mod.get_axon_ntff_profile_hook = _get
        sys.modules["antenv.axon_hooks"] = hooks_mod
        antenv.axon_hooks = hooks_mod
    if hooks_mod.get_axon_ntff_profile_hook() is None:
        try:
            from trn_agent_boot.trn_boot import _ntff_profile_via_ctypes
            hook = _ntff_profile_via_ctypes("/opt/axon/libaxon_pjrt.so")
            if hook is not None:
                hooks_mod.set_axon_ntff_profile_hook(hook)
        except Exception:
            pass


def _prep_pass(q, db, grid, nb_core):
    """Layout + candidate gather + per-core packing for one KNN pass."""
    qperm, lo, hi = _layout3d(q, grid, RADIUS)
    cands = _block_candidates(db, lo, hi)
    nblocks = len(cands)
    nts0 = np.array([-(-len(c) // T) for c in cands])
    # greedy slot balancing: largest blocks first into the least-loaded core
    # (capped at nb_core blocks per core so the SPMD slot streams line up)
    order = np.argsort(-nts0, kind="stable")
    loads = [0] * CORES
    counts = [0] * CORES
    blists = [[] for _ in range(CORES)]
    for b in order:
        c = min((i for i in range(CORES) if counts[i] < nb_core),
                key=lambda i: loads[i])
        blists[c].append(int(b))
        loads[c] += int(nts0[b])
        counts[c] += 1
    for c in range(CORES):
        blists[c].sort()
    nw = -(-max(loads) // SLOTS)

    qf = np.ascontiguousarray(np.stack([
        sq_query(q)[qperm].astype(np.float32),
        np.ones(len(qperm), np.float32),
        _f32(-2.0 * q[qperm, 0]), _f32(-2.0 * q[qperm, 1]), _f32(-2.0 * q[qperm, 2]),
    ]).astype(np.float32))
    df = np.ascontiguousarray(np.stack([
        np.ones(db.shape[0], np.float32), sq_db(db).astype(np.float32),
        db[:, 0], db[:, 1], db[:, 2]]).astype(np.float32))

    cores = []
    ntmax = int(nts0.max())
    gidx_full = np.zeros((nblocks, ntmax * T), np.int64)
    nts_full = np.zeros(nblocks, np.int64)
    for c in range(CORES):
        dwarr, slot_blocks, gidx_rows, nts = _pack_core(cands, df, qf, blists[c], nw)
        cores.append((dwarr, slot_blocks))
        for bi, b in enumerate(blists[c]):
            gidx_full[b, : len(gidx_rows[bi])] = gidx_rows[bi]
            nts_full[b] = nts[bi]
    return dict(qperm=qperm, cores=cores, gidx=gidx_full, nts=nts_full,
                ntmax=ntmax, nw=nw, blists=blists)


def _assemble_tmin(tm_cores, slot_blocks_cores, blists, nts, ntmax):
    """Map device per-slot minima back to [Rn, NTmax] (inf-padded)."""
    nblocks = len(nts)
    out = np.full((nblocks * 128, ntmax), np.inf, np.float32)
    for c, tmo in enumerate(tm_cores):
        nw = tmo.shape[1] // SLOTS
        sb = slot_blocks_cores[c]
        blist = blists[c]
        tile_seen = {}
        for i in range(nw * SLOTS):
            bi = int(sb[i])
            t = tile_seen.get(bi, 0)
            b = blist[bi]
            if t < nts[b]:
                w, s = divmod(i, SLOTS)
                out[b * 128:(b + 1) * 128, t] = tmo[:, w * SLOTS + OUTCOL[s]]
                tile_seen[bi] = t + 1
    return out


def kernel(pred_xyz, tgt_xyz, tgt_rgb, search_range):
    global _LAST_RESULTS
    from concourse.bass_utils import run_bass_kernel_spmd

    pred = np.ascontiguousarray(np.asarray(pred_xyz, dtype=np.float32))
    tgt = np.ascontiguousarray(np.asarray(tgt_xyz, dtype=np.float32))
    rgb = np.ascontiguousarray(np.asarray(tgt_rgb, dtype=np.float32))
    assert pred.shape == (M, 3) and tgt.shape == (N, 3)

    nbb = (N // 128) // CORES   # 48 backward blocks per core
    nbf = (M // 128) // CORES   # 64 forward blocks per core

    B = _prep_pass(tgt, pred, GRID_B, nbb)    # backward: queries tgt, db pred
    F = _prep_pass(pred, tgt, GRID_F, nbf)    # forward: queries pred, db tgt

    in_maps = []
    for c in range(CORES):
        in_maps.append({
            "dwb": B["cores"][c][0],
            "dwf": F["cores"][c][0],
        })

    nc = _build_program(B["nw"], F["nw"])
    trace = bool(int(os.environ.get("KNN_TRACE", "0")))
    if trace:
        _install_ntff_hook()
    try:
        res = run_bass_kernel_spmd(nc, in_maps, core_ids=list(range(CORES)), trace=trace)
    except Exception:
        if not trace:
            raise
        res = run_bass_kernel_spmd(nc, in_maps, core_ids=list(range(CORES)), trace=False)
    _LAST_RESULTS = res

    tmin_b = _assemble_tmin([res.results[c]["tmb"] for c in range(CORES)],
                            [B["cores"][c][1] for c in range(CORES)],
                            B["blists"], B["nts"], B["ntmax"])
    tmin_f = _assemble_tmin([res.results[c]["tmf"] for c in range(CORES)],
                            [F["cores"][c][1] for c in range(CORES)],
                            F["blists"], F["nts"], F["ntmax"])

    bmin, bties = _knn_pass(tgt, pred, tmin_b, B["gidx"], B["qperm"], RADIUS * RADIUS)
    fmin, fties = _knn_pass(pred, tgt, tmin_f, F["gidx"], F["qperm"], RADIUS * RADIUS)

    return _finish(pred, tgt, rgb, bmin, bties, fmin, fties)
